# revision 1
# baseline (speedup 1.0000x reference)
"""AdMSoftmaxLoss distributed Trainium2 kernel.

Reference computation (N=8192, D=1024, C=10240, S=30, ml=0.4, ms=0.1):
    wf    = clip(l2norm(x) @ l2norm(weight).T, -1, 1)      # (N, C) cosines
    m     = where(labels <= 5, ml, ms)
    t     = wf[i, labels[i]]
    num   = S * (t - m)
    excl  = sum_j exp(S * wf[i, j]) - exp(S * t)
    L     = num - log(exp(num) + excl)
    loss  = -mean(L)

Sharding: 2 row-groups x 4 class-groups over 8 NeuronCores. Core i gets
rows [ (i//4)*4096, .. ) and classes [ (i%4)*2560, .. ). Each core
computes, for its (row, class) block:
    out[0][r] = sum_{c in block} exp(S * cos[r, c])       (partial denom)
    out[1][r] = exp(S * cos[r, labels[r]]) if label owned  (partial)
The host sums partials over class groups, recovers t = log(out1)/S, and
finishes the O(N) loss arithmetic (one million times less work than the
device-side matmul).

Device pipeline per core:
  - weight (per 512-class chunk): HWDGE f32 load; row sum-of-squares on
    VectorE (square+accum fused in one scalar_tensor_tensor, bf16 out
    for 2x mode); 1/||w|| via batched Newton rsqrt on VectorE (fixed
    seed, 3 iterations - row norms are tightly distributed); normalize +
    cast f32->bf16 fused in one tensor_scalar; write to DRAM scratch;
    DMA-xbar transpose -> wnT (d-major).
  - x (per 128-row tile): HWDGE f32 load; ScalarE Copy cast to bf16
    (Copy lives in every ACT table set); VectorE square+accum; Newton
    rsqrt batched per 8-tile group with the 30x folded into the last
    iteration: the ScalarE exp applies scale 30/||x|| per partition, so
    x stays unnormalized and matmul computes dot(x16, wn16) = cos*||x||.
    ScalarE runs ONLY Exp + Copy -> a single ACT table load.
  - matmul: 32 m-tiles x 5 n-chunks(512) x 8 k-tiles, bf16, PSUM f32.
  - epilogue per superchunk (512/1024/1024 classes): ScalarE Exp with
    accum_out (fused row-sum); VectorE scalar_tensor_tensor
    (iota == label-offset) * exp with accum_out (fused label gather).
  - all prep for group g+1 / later weight chunks is emitted as small
    background tasks BETWEEN epilogue steps of group g, keeping every
    engine's program stream free of long head-of-line waits.
"""

import math
import os
import numpy as np

P = 128
N_ROWS, D, C = 8192, 1024, 10240
S = 30.0
ML, MS = 0.4, 0.1
NCORES = 8
RG, CG = 2, 4                  # row groups x class groups
R_LOC = N_ROWS // RG           # 4096
C_LOC = C // CG                # 2560
M_TILES = R_LOC // P           # 32
NCHUNK = 512
N_CHUNKS = C_LOC // NCHUNK     # 5
# epilogue superchunks (start, width); first narrow so MMs start after
# only one weight chunk is ready
SUPER = [(0, 512), (512, 1024), (1536, 1024)]
K_TILES = D // P               # 8
W_PER_CHUNK = NCHUNK // P      # 4 weight 128-row tiles per n-chunk
GROUPS = 4                     # x prep/transpose pipeline groups
G_MT = M_TILES // GROUPS       # 8 m-tiles per group
G_ROWS = R_LOC // GROUPS       # 1024

# Fixed Newton rsqrt seeds: x rows ~ chi2(1024) -> ns ~= 1024;
# xavier weight rows -> ns ~= D * limit^2 / 3 = 2*D/(C+D) = 0.182
R0_X = 1.0 / math.sqrt(1024.0)
R0_W = 1.0 / math.sqrt(2.0 * D / (C + D))

_CACHE = {}
LAST_RESULTS = None  # BassKernelResults of the most recent run (for test.py)


def _build():
    """Build + compile the SPMD Bass graph once; cache in module global."""
    if "nc" in _CACHE:
        return _CACHE["nc"]

    import concourse.bass as bass
    import concourse.mybir as mybir
    import concourse.tile as tile
    from concourse import bacc

    ts = bass.ts
    dt = mybir.dt
    AF = mybir.ActivationFunctionType
    ALU = mybir.AluOpType

    nc = bacc.Bacc(
        "TRN2", target_bir_lowering=False, debug=False, num_devices=NCORES
    )

    x_ext = nc.dram_tensor("x", [R_LOC, D], dt.float32, kind="ExternalInput").ap()
    w_ext = nc.dram_tensor("w", [C_LOC, D], dt.float32, kind="ExternalInput").ap()
    lab_ext = nc.dram_tensor(
        "lab", [P, M_TILES], dt.float32, kind="ExternalInput"
    ).ap()
    iota_ext = nc.dram_tensor(
        "iota", [P, C_LOC], dt.float32, kind="ExternalInput"
    ).ap()
    ident_ext = nc.dram_tensor(
        "ident", [P, P], dt.bfloat16, kind="ExternalInput"
    ).ap()
    out_ext = nc.dram_tensor(
        "out", [3, P, M_TILES], dt.float32, kind="ExternalOutput"
    ).ap()

    with tile.TileContext(nc) as tc:
        with (
            tc.tile_pool(name="dram", bufs=1, space="DRAM") as dram,
            tc.tile_pool(name="consts", bufs=1) as consts,
            tc.tile_pool(name="wstage", bufs=4) as wstage,
            tc.tile_pool(name="wgrp", bufs=2) as wgrp,
            tc.tile_pool(name="xstage", bufs=3) as xstage,
            tc.tile_pool(name="xgrp", bufs=2) as xgrp,
            tc.tile_pool(name="sq", bufs=4) as sqpool,
            tc.tile_pool(name="small", bufs=8) as small,
            tc.tile_pool(name="gacc", bufs=2) as gacc,
            tc.tile_pool(name="xnt", bufs=2) as xnt_pool,
            tc.tile_pool(name="epi", bufs=2) as epi,
            tc.tile_pool(name="epi2", bufs=2) as epi2,
            tc.tile_pool(name="psum", bufs=3, space="PSUM") as psum,
            tc.tile_pool(name="psumt", bufs=2, space="PSUM") as psumt,
        ):
            xb_dram = dram.tile([R_LOC, D], dt.bfloat16)
            wb_dram = dram.tile([C_LOC, D], dt.bfloat16)

            iota_sb = consts.tile([P, C_LOC], dt.float32)
            nc.sync.dma_start(iota_sb[:], iota_ext)
            ident_sb = consts.tile([P, P], dt.bfloat16)
            nc.scalar.dma_start(ident_sb[:], ident_ext)
            lab_sb = consts.tile([P, M_TILES], dt.float32)
            nc.sync.dma_start(lab_sb[:], lab_ext)

            outsum = consts.tile([P, M_TILES], dt.float32)
            outtgt = consts.tile([P, M_TILES], dt.float32)
            outscl = consts.tile([P, M_TILES], dt.float32)

            # wnT[n][d_partition, k, class-in-chunk] : d-major weight, fp8
            # (x16 scale baked into 1/||w||; un-scaled in the exp scale)
            wnT = [
                consts.tile([P, K_TILES, NCHUNK], dt.float8e4, name=f"wnT{n}", tag=f"wnT{n}")
                for n in range(N_CHUNKS)
            ]

            def newton_rsqrt(ns, r, scale_last=1.0):
                """r <- scale_last / sqrt(ns), elementwise, 3 Newton steps.

                ns, r: (P, B) f32 tiles; r pre-filled with the seed.
                """
                B = ns.shape[-1]
                for it in range(2):
                    a = small.tile([P, 8], dt.float32, tag="nw_a")
                    nc.vector.scalar_tensor_tensor(
                        a[:, :B], r, 1.0, r, op0=ALU.mult, op1=ALU.mult
                    )  # r^2
                    b = small.tile([P, 8], dt.float32, tag="nw_b")
                    nc.vector.scalar_tensor_tensor(
                        b[:, :B], a[:, :B], 1.0, ns, op0=ALU.mult, op1=ALU.mult
                    )  # ns * r^2
                    c = small.tile([P, 8], dt.float32, tag="nw_c")
                    s = scale_last if it == 1 else 1.0
                    nc.vector.tensor_scalar(
                        c[:, :B], b[:, :B], -0.5 * s, 1.5 * s, ALU.mult, ALU.add
                    )  # s*(1.5 - 0.5 ns r^2)
                    r2 = small.tile([P, 8], dt.float32, tag="nw_r")
                    nc.vector.scalar_tensor_tensor(
                        r2[:, :B], r, 1.0, c[:, :B], op0=ALU.mult, op1=ALU.mult
                    )
                    r = r2[:, :B]
                return r

            def prep_w_chunk_a(n):
                """Load + normalize 512 weight rows of n-chunk n (compute)."""
                wns = small.tile([P, W_PER_CHUNK], dt.float32, tag="wns")
                wnrm = []
                for wi in range(W_PER_CHUNK):
                    wt = n * W_PER_CHUNK + wi
                    wtile = wstage.tile([P, D], dt.float32, tag="wtile")
                    nc.scalar.dma_start(wtile[:], w_ext[ts(wt, P), :])
                    sq = sqpool.tile([P, D], dt.bfloat16, tag="sq")
                    nc.vector.scalar_tensor_tensor(
                        sq[:],
                        wtile[:],
                        1.0,
                        wtile[:],
                        op0=ALU.mult,
                        op1=ALU.mult,
                        accum_out=wns[:, wi : wi + 1],
                    )
                    wnrm.append(wtile)
                rw = small.tile([P, W_PER_CHUNK], dt.float32, tag="wr0")
                nc.gpsimd.memset(rw[:], R0_W)
                winv = newton_rsqrt(wns[:], rw[:], scale_last=16.0)
                wn = wgrp.tile([P, W_PER_CHUNK, D], dt.bfloat16, tag="wn")
                for wi in range(W_PER_CHUNK):
                    # normalize + cast f32 -> bf16 in one pass
                    nc.vector.tensor_scalar_mul(
                        wn[:, wi, :], wnrm[wi][:], winv[:, wi : wi + 1]
                    )
                return wn

            def prep_w_chunk_b_pe(n, wn):
                """Transpose chunk n into wnT on the (idle) TensorEngine."""
                for wi in range(W_PER_CHUNK):
                    tp = psumt.tile([P, K_TILES, P], dt.bfloat16, tag="tps")
                    for k in range(K_TILES):
                        nc.tensor.transpose(
                            tp[:, k, :], wn[:, wi, ts(k, P)], ident_sb[:]
                        )
                    nc.scalar.copy(wnT[n][:, :, ts(wi, P)], tp[:])

            def prep_w_chunk_b(n, wn):
                """Write back + transpose n-chunk n into wnT."""
                nc.sync.dma_start(
                    wb_dram[ts(n, NCHUNK), :].rearrange(
                        "(wi p) d -> p wi d", p=P
                    ),
                    wn[:],
                )
                for k in range(K_TILES):
                    nc.sync.dma_start_transpose(
                        wnT[n][:, k, :],
                        wb_dram[ts(n, NCHUNK), ts(k, P)],
                    )

            def prep_w_chunk(n):
                prep_w_chunk_b(n, prep_w_chunk_a(n))

            def make_x_group_tasks(g, state, pe_transpose=False):
                """Closures: 8 per-tile preps + 2 finalize (wb+transpose)."""
                xns = small.tile([P, G_MT], dt.float32, tag="xns")
                xts = xgrp.tile([P, G_MT, D], dt.bfloat16, tag="xts")
                if pe_transpose:
                    xnT_pe = xnt_pool.tile(
                        [P, K_TILES, G_ROWS], dt.float8e4, tag="xnT8"
                    )

                def pair_task(jj):
                    def run():
                        m = g * G_MT + 2 * jj
                        xf2 = xstage.tile([P, 2, D], dt.float32, tag="xf2")
                        nc.scalar.dma_start(
                            xf2[:],
                            x_ext[m * P : (m + 2) * P, :].rearrange(
                                "(j p) d -> p j d", p=P
                            ),
                        )
                        for s in range(2):
                            j = 2 * jj + s
                            xt = xts[:, j, :]
                            nc.scalar.copy(xt, xf2[:, s, :])  # cast to bf16
                            sqx = sqpool.tile([P, D], dt.bfloat16, tag="sq")
                            nc.vector.scalar_tensor_tensor(
                                sqx[:],
                                xt,
                                1.0,
                                xt,
                                op0=ALU.mult,
                                op1=ALU.mult,
                                accum_out=xns[:, j : j + 1],
                            )
                            if pe_transpose:
                                tp = psumt.tile(
                                    [P, K_TILES, P], dt.bfloat16, tag="tps"
                                )
                                for k in range(K_TILES):
                                    nc.tensor.transpose(
                                        tp[:, k, :],
                                        xts[:, j, ts(k, P)],
                                        ident_sb[:],
                                    )
                                if g % 2 == 0:
                                    nc.vector.tensor_copy(
                                        xnT_pe[:, :, ts(j, P)], tp[:]
                                    )
                                else:
                                    nc.scalar.copy(
                                        xnT_pe[:, :, ts(j, P)], tp[:]
                                    )

                    return run

                def final_task_a():
                    rx = small.tile([P, G_MT], dt.float32, tag="xr0")
                    nc.gpsimd.memset(rx[:], R0_X)
                    scl30 = newton_rsqrt(xns[:], rx[:], scale_last=S / 16.0)
                    if pe_transpose:
                        state[g] = (scl30, xnT_pe, None)
                        return
                    # one batched writeback for the whole group
                    nc.sync.dma_start(
                        xb_dram[ts(g, G_ROWS), :].rearrange(
                            "(j p) d -> p j d", p=P
                        ),
                        xts[:],
                    )
                    xnTb = xnt_pool.tile(
                        [P, K_TILES, G_ROWS], dt.bfloat16, tag="xnTb"
                    )
                    xnT8 = xnt_pool.tile(
                        [P, K_TILES, G_ROWS], dt.float8e4, tag="xnT8"
                    )
                    for k in range(K_TILES // 2):
                        nc.sync.dma_start_transpose(
                            xnTb[:, k, :], xb_dram[ts(g, G_ROWS), ts(k, P)]
                        )
                    nc.scalar.copy(
                        xnT8[:, : K_TILES // 2, :], xnTb[:, : K_TILES // 2, :]
                    )
                    state[g] = (scl30, xnT8, xnTb)

                def final_task_b():
                    if pe_transpose:
                        return
                    scl30, xnT8, xnTb = state[g]
                    for k in range(K_TILES // 2, K_TILES):
                        nc.sync.dma_start_transpose(
                            xnTb[:, k, :], xb_dram[ts(g, G_ROWS), ts(k, P)]
                        )
                    nc.scalar.copy(
                        xnT8[:, K_TILES // 2 :, :], xnTb[:, K_TILES // 2 :, :]
                    )

                return [pair_task(jj) for jj in range(G_MT // 2)] + [
                    final_task_a,
                    final_task_b,
                ]

            def mm_superchunk(xnT, j, c0, width):
                ps = psum.tile([P, 1024], dt.float32, tag="ps")
                for kp in range(K_TILES // 2):
                    for h in range(width // NCHUNK):
                        n = (c0 + h * NCHUNK) // NCHUNK
                        nc.tensor.matmul(
                            ps[:, ts(h, NCHUNK)],
                            xnT[:, 2 * kp : 2 * kp + 2, ts(j, P)],
                            wnT[n][:, 2 * kp : 2 * kp + 2, :],
                            start=(kp == 0),
                            stop=(kp == K_TILES // 2 - 1),
                            perf_mode=mybir.MatmulPerfMode.DoubleRow,
                        )
                return ps

            def run_group(g, state, tasks):
                """Matmuls + epilogue for row group g; interleave bg tasks."""
                scl30, xnT, _xb = state[g]
                nsc = len(SUPER)
                sums = gacc.tile([P, G_MT, nsc], dt.float32, tag="sums")
                if g == 0:
                    # si-outer (weight chunks still streaming in):
                    # 3 narrow gathers per m-tile
                    tgts = gacc.tile([P, G_MT, nsc], dt.float32, tag="tgts")
                    for si, (c0, width) in enumerate(SUPER):
                        for j in range(G_MT):
                            m = g * G_MT + j
                            if tasks:
                                tasks.pop(0)()
                            ps = mm_superchunk(xnT, j, c0, width)
                            esc = epi.tile([P, 1024], dt.float32, tag="esc")
                            nc.scalar.activation(
                                esc[:, :width],
                                ps[:, :width],
                                AF.Exp,
                                scale=scl30[:, j : j + 1],
                                accum_out=sums[:, j, si : si + 1],
                            )
                            msc = epi.tile([P, 1024], dt.float32, tag="msc")
                            nc.vector.scalar_tensor_tensor(
                                msc[:, :width],
                                iota_sb[:, c0 : c0 + width],
                                lab_sb[:, m : m + 1],
                                esc[:, :width],
                                op0=ALU.is_equal,
                                op1=ALU.mult,
                                accum_out=tgts[:, j, si : si + 1],
                            )
                    nc.vector.tensor_reduce(
                        outtgt[:, ts(g, G_MT)],
                        tgts[:],
                        axis=mybir.AxisListType.X,
                        op=ALU.add,
                    )
                else:
                    # j-outer: one full-width gather per m-tile, accumulated
                    # straight into the output column
                    for j in range(G_MT):
                        m = g * G_MT + j
                        escf = epi2.tile([P, C_LOC], dt.float32, tag="escf")
                        for si, (c0, width) in enumerate(SUPER):
                            if tasks:
                                tasks.pop(0)()
                            ps = mm_superchunk(xnT, j, c0, width)
                            nc.scalar.activation(
                                escf[:, c0 : c0 + width],
                                ps[:, :width],
                                AF.Exp,
                                scale=scl30[:, j : j + 1],
                                accum_out=sums[:, j, si : si + 1],
                            )
                        mscf = epi2.tile([P, C_LOC], dt.float32, tag="mscf")
                        nc.vector.scalar_tensor_tensor(
                            mscf[:],
                            iota_sb[:],
                            lab_sb[:, m : m + 1],
                            escf[:],
                            op0=ALU.is_equal,
                            op1=ALU.mult,
                            accum_out=outtgt[:, m : m + 1],
                        )
                nc.vector.tensor_reduce(
                    outsum[:, ts(g, G_MT)],
                    sums[:],
                    axis=mybir.AxisListType.X,
                    op=ALU.add,
                )
                nc.vector.tensor_copy(outscl[:, ts(g, G_MT)], scl30)

            state = {}
            # startup: interleave w chunks 0-2 with x group 0 so the scalar
            # ring streams all loads while DVE/ACT pipeline the prep
            x0 = make_x_group_tasks(0, state, pe_transpose=True)
            w0 = prep_w_chunk_a(0)
            for t in x0[:4]:
                t()
            prep_w_chunk_b_pe(0, w0)
            x0[4]()  # newton -> scl30 for group 0
            x0[5]()
            wpend = {1: prep_w_chunk_a(1), 2: prep_w_chunk_a(2)}
            for g in range(GROUPS):
                tasks = []
                if g == 0:
                    xp = make_x_group_tasks(1, state, pe_transpose=True)

                    def wa(n):
                        def f():
                            wpend[n] = prep_w_chunk_a(n)

                        return f

                    def wb(n):
                        def f():
                            prep_w_chunk_b_pe(n, wpend.pop(n))

                        return f

                    def wbd(n):
                        def f():
                            prep_w_chunk_b(n, wpend.pop(n))

                        return f

                    tasks = [
                        wb(1), xp[0], wb(2), wa(3),
                        xp[1], wb(3), wa(4), xp[2],
                        wb(4), xp[3], xp[4], xp[5],
                    ]
                elif g + 1 < GROUPS:
                    tasks = make_x_group_tasks(g + 1, state, pe_transpose=True)
                run_group(g, state, tasks)

            nc.sync.dma_start(out_ext[0], outsum[:])
            nc.sync.dma_start(out_ext[1], outtgt[:])
            nc.sync.dma_start(out_ext[2], outscl[:])

    nc.compile()
    _CACHE["nc"] = nc
    return nc


def _make_in_maps(x, labels, weight):
    import ml_dtypes

    iota = np.broadcast_to(
        np.arange(C_LOC, dtype=np.float32)[None, :], (P, C_LOC)
    ).copy()
    ident = np.eye(P, dtype=ml_dtypes.bfloat16)
    labels_f = labels.astype(np.float32)
    in_maps = []
    for i in range(NCORES):
        gr, ci = divmod(i, CG)
        xs = np.ascontiguousarray(x[gr * R_LOC : (gr + 1) * R_LOC])
        ws = np.ascontiguousarray(weight[ci * C_LOC : (ci + 1) * C_LOC])
        lab = labels_f[gr * R_LOC : (gr + 1) * R_LOC] - ci * C_LOC
        lab_shuf = np.ascontiguousarray(lab.reshape(M_TILES, P).T)
        in_maps.append(
            {
                "x": xs,
                "w": ws,
                "lab": lab_shuf,
                "iota": iota,
                "ident": ident,
            }
        )
    return in_maps


def kernel(x, labels, weight):
    global LAST_RESULTS
    from concourse.bass_utils import run_bass_kernel_spmd

    x = np.asarray(x, dtype=np.float32)
    weight = np.asarray(weight, dtype=np.float32)
    labels = np.asarray(labels)

    nc = _build()
    in_maps = _make_in_maps(x, labels, weight)
    trace = bool(int(os.environ.get("ADMS_TRACE", "0")))
    res = run_bass_kernel_spmd(
        nc, in_maps, list(range(NCORES)), trace=trace
    )
    LAST_RESULTS = res

    total = np.zeros(N_ROWS, np.float64)
    tgtraw = np.zeros(N_ROWS, np.float64)
    scl = np.zeros(N_ROWS, np.float64)
    for i, r in enumerate(res.results):
        gr = i // CG
        o = np.asarray(r["out"], dtype=np.float64).reshape(3, P, M_TILES)
        part = o.transpose(0, 2, 1).reshape(3, R_LOC)  # [s, m*P + p]
        sl = slice(gr * R_LOC, (gr + 1) * R_LOC)
        total[sl] += part[0]
        tgtraw[sl] += part[1]
        scl[sl] = part[2]

    t = np.log(tgtraw) / S  # out[1] = exp(S*cos_label) partials
    t = np.clip(t, -1.0, 1.0)
    m = np.where(labels <= 5, ML, MS)
    num = S * (t - m)
    L = num - np.log(np.exp(num) + (total - np.exp(S * t)))
    return np.float32(-L.mean())



# revision 5
# speedup vs baseline: 1.7411x; 1.7411x over previous
"""AdMSoftmaxLoss distributed Trainium2 kernel (v2 — host-prepped operands).

Reference computation (N=8192, D=1024, C=10240, S=30, ml=0.4, ms=0.1):
    wf    = clip(l2norm(x) @ l2norm(weight).T, -1, 1)      # (N, C) cosines
    m     = where(labels <= 5, ml, ms)
    t     = wf[i, labels[i]]
    num   = S * (t - m)
    excl  = sum_j exp(S * wf[i, j]) - exp(S * t)
    L     = num - log(exp(num) + excl)
    loss  = -mean(L)

Sharding: 2 row-groups x 4 class-groups over 8 NeuronCores. Core i gets
rows [ (i//4)*4096, .. ) and classes [ (i%4)*2560, .. ).

Division of labor:
  - HOST (numpy, ~1e7 elem ops, 1e4x less work than the device matmul):
    l2-normalize x and weight, scale by 16, cast to fp8e4m3, and lay the
    operands out d-major (pre-transposed) exactly as the PE wants them.
    Also computes the per-row label term t = cos(x_i, w_label) exactly,
    which replaces both the device-side label gather and the all-reduce.
  - DEVICE: for its (4096 rows x 2560 classes) block, computes
    out[p, m] = sum_c exp(S * cos[row, c]) via fp8 DoubleRow matmuls
    (contraction 256/pass, 4 passes over D=1024) and ScalarE Exp with
    fused row-sum accumulation. That is the only O(N*C) work.
  - HOST finish: total denominator = sum of 4 class-group partials,
    excl = total - exp(S*t_q), L = num - log(exp(num) + excl), mean.

Device pipeline per core: DMA fp8 operands in 7 chunks ordered so the
first matmul can start after ~1.5MB has landed; 640 DoubleRow matmuls
(N=512 each, PSUM f32, 3 rotating 2-bank accumulators); 96 Exp
activations with accum_out; one tensor_reduce; 16KB output DMA.
"""

import math
import os
import numpy as np

P = 128
N_ROWS, D, C = 8192, 1024, 10240
S = 30.0
ML, MS = 0.4, 0.1
NCORES = 8
RG, CG = 2, 4                  # row groups x class groups
R_LOC = N_ROWS // RG           # 4096
C_LOC = C // CG                # 2560
M_TILES = R_LOC // P           # 32
K_TILES = D // P               # 8
KP = K_TILES // 2              # 4 DoubleRow passes (256 contraction each)
XCH = 4                        # x row chunks (1024 rows each)
XW = R_LOC // XCH              # 1024
G_MT = XW // P                 # 8 m-tiles per x chunk
SUPER = [(0, 512), (512, 1024), (1536, 1024)]   # class superchunks
NSC = len(SUPER)
FS = 16.0                      # fp8 pre-scale on both operands
EXPSCALE = S / (FS * FS)       # PSUM holds FS^2 * cos

_CACHE = {}
LAST_RESULTS = None  # BassKernelResults of the most recent run (for test.py)


def _build():
    """Build + compile the SPMD Bass graph once; cache in module global."""
    if "nc" in _CACHE:
        return _CACHE["nc"]

    import concourse.bass as bass
    import concourse.mybir as mybir
    import concourse.tile as tile
    from concourse import bacc

    ts = bass.ts
    dt = mybir.dt
    AF = mybir.ActivationFunctionType
    ALU = mybir.AluOpType

    nc = bacc.Bacc(
        "TRN2", target_bir_lowering=False, debug=False, num_devices=NCORES
    )

    x_ext = nc.dram_tensor(
        "xq", [XCH, P, K_TILES, XW], dt.float8e4, kind="ExternalInput"
    ).ap()
    w_exts = [
        nc.dram_tensor(
            f"wq{si}", [P, K_TILES, w], dt.float8e4, kind="ExternalInput"
        ).ap()
        for si, (c0, w) in enumerate(SUPER)
    ]
    out_ext = nc.dram_tensor(
        "out", [XCH, P, G_MT], dt.float32, kind="ExternalOutput"
    ).ap()

    with tile.TileContext(nc) as tc:
        with (
            tc.tile_pool(name="consts", bufs=1) as consts,
            tc.tile_pool(name="esc", bufs=3) as escp,
            tc.tile_pool(name="psum", bufs=3, space="PSUM") as psum,
            tc.tile_pool(name="psumw", bufs=1, space="PSUM") as psumw,
        ):
            wsb = [
                consts.tile([P, K_TILES, w], dt.float8e4, name=f"w{si}", tag=f"w{si}")
                for si, (c0, w) in enumerate(SUPER)
            ]
            xsb = [
                consts.tile([P, K_TILES, XW], dt.float8e4, name=f"x{g}", tag=f"x{g}")
                for g in range(XCH)
            ]
            sums = consts.tile([P, M_TILES, NSC], dt.float32)
            outsum = consts.tile([P, M_TILES], dt.float32)

            # DMA order = need order. sync ring: w chunk 0, x chunk 0 (in
            # two halves so the first matmuls start sooner), then x rest;
            # scalar ring (second HWDGE): the later w chunks in parallel.
            nc.sync.dma_start(wsb[0][:], w_exts[0])
            nc.sync.dma_start(xsb[0][:, :, 0:512], x_ext[0][:, :, 0:512])
            nc.sync.dma_start(xsb[0][:, :, 512:XW], x_ext[0][:, :, 512:XW])
            nc.scalar.dma_start(wsb[1][:], w_exts[1])
            nc.scalar.dma_start(wsb[2][:], w_exts[2])
            nc.sync.dma_start(xsb[1][:], x_ext[1])
            nc.sync.dma_start(xsb[2][:], x_ext[2])
            nc.sync.dma_start(xsb[3][:], x_ext[3])

            # Warm the PE HAM clock gate while the first chunks stream in:
            # a dozen throwaway matmuls on a zeroed tile, so the first
            # real matmuls run at 2.4 GHz instead of 1.2.
            zf = consts.tile([P, 2, 512], dt.float8e4)
            nc.gpsimd.memset(zf[:], 0.0)
            zps = psumw.tile([P, 512], dt.float32)
            for _ in range(12):
                nc.tensor.matmul(
                    zps[:],
                    zf[:, :, 0:P],
                    zf[:],
                    start=True,
                    stop=True,
                    perf_mode=mybir.MatmulPerfMode.DoubleRow,
                )

            def block(g, jj, si):
                """Matmuls + exp row-sum for (m-tile, superchunk)."""
                m = g * G_MT + jj
                _, w = SUPER[si]
                ps = psum.tile([P, 1024], dt.float32, tag="ps")
                for kp in range(KP):
                    for h in range(w // 512):
                        nc.tensor.matmul(
                            ps[:, ts(h, 512)],
                            xsb[g][:, 2 * kp : 2 * kp + 2, ts(jj, P)],
                            wsb[si][:, 2 * kp : 2 * kp + 2, ts(h, 512)],
                            start=(kp == 0),
                            stop=(kp == KP - 1),
                            perf_mode=mybir.MatmulPerfMode.DoubleRow,
                        )
                esc = escp.tile([P, 1024], dt.bfloat16, tag="esc")
                nc.scalar.activation(
                    esc[:, :w],
                    ps[:, :w],
                    AF.Exp,
                    scale=EXPSCALE,
                    accum_out=sums[:, m, si : si + 1],
                )

            for g in range(XCH):
                if g == 0:
                    # si-major startup: row chunk 0 runs all 8 m-tiles of
                    # superchunk 0 first (jj 0-3 before 4-7 so only the
                    # first half of xq[0] gates the first matmul).
                    for si in range(NSC):
                        for jj in range(G_MT):
                            block(g, jj, si)
                else:
                    for jj in range(G_MT):
                        for si in range(NSC):
                            block(g, jj, si)
                # flush this row group's partials so only the last
                # group's reduce + DMA sit on the critical-path tail
                nc.vector.tensor_reduce(
                    outsum[:, ts(g, G_MT)],
                    sums[:, ts(g, G_MT), :],
                    axis=mybir.AxisListType.X,
                    op=ALU.add,
                )
                nc.sync.dma_start(out_ext[g], outsum[:, ts(g, G_MT)])

    nc.compile()
    _CACHE["nc"] = nc
    return nc


def _prep_inputs(x, weight):
    """Normalize, scale, fp8-quantize, and transpose operands host-side.

    Returns (in_maps_pieces, xq_f32, wq_f32) where the f32 copies of the
    quantized values are used to reproduce the device's label term.
    """
    import ml_dtypes

    f8 = ml_dtypes.float8_e4m3

    xn = x / np.maximum(np.sqrt((x * x).sum(1, keepdims=True)), 1e-12)
    wn = weight / np.maximum(np.sqrt((weight * weight).sum(1, keepdims=True)), 1e-12)
    xq = (xn * FS).astype(f8)
    wq = (wn * FS).astype(f8)

    x_groups = []
    for gr in range(RG):
        xg = xq[gr * R_LOC : (gr + 1) * R_LOC]          # [4096, 1024]
        # A[c, p, k, r] = xg.T[k*128+p, c*1024+r]
        a = np.ascontiguousarray(
            xg.T.reshape(K_TILES, P, XCH, XW).transpose(2, 1, 0, 3)
        )
        x_groups.append(a)

    w_chunks = []
    for ci in range(CG):
        wc = wq[ci * C_LOC : (ci + 1) * C_LOC]           # [2560, 1024]
        wt = wc.T.reshape(K_TILES, P, C_LOC).transpose(1, 0, 2)  # [p, k, c]
        w_chunks.append(
            [np.ascontiguousarray(wt[:, :, c0 : c0 + w]) for c0, w in SUPER]
        )

    return x_groups, w_chunks, xq, wq


def kernel(x, labels, weight):
    global LAST_RESULTS
    from concourse.bass_utils import run_bass_kernel_spmd

    x = np.asarray(x, dtype=np.float32)
    weight = np.asarray(weight, dtype=np.float32)
    labels = np.asarray(labels).astype(np.int64)

    nc = _build()
    x_groups, w_chunks, xq, wq = _prep_inputs(x, weight)

    in_maps = []
    for i in range(NCORES):
        gr, ci = divmod(i, CG)
        im = {"xq": x_groups[gr]}
        for si in range(NSC):
            im[f"wq{si}"] = w_chunks[ci][si]
        in_maps.append(im)

    trace = bool(int(os.environ.get("ADMS_TRACE", "0")))
    res = run_bass_kernel_spmd(nc, in_maps, list(range(NCORES)), trace=trace)
    LAST_RESULTS = res

    total = np.zeros(N_ROWS, np.float64)
    for i, r in enumerate(res.results):
        gr = i // CG
        o = np.asarray(r["out"], dtype=np.float64)       # [4, 128, 8]
        # row = g*1024 + jj*128 + p  ->  [g, jj, p] flat
        part = o.transpose(0, 2, 1).reshape(R_LOC)
        total[gr * R_LOC : (gr + 1) * R_LOC] += part

    # Label term: exact for the numerator; quantized (matching the
    # device's fp8 operands) for the excl subtraction.
    xn = x.astype(np.float64)
    xn /= np.maximum(np.sqrt((xn * xn).sum(1, keepdims=True)), 1e-12)
    wn_lab = weight[labels].astype(np.float64)
    wn_lab /= np.maximum(np.sqrt((wn_lab * wn_lab).sum(1, keepdims=True)), 1e-12)
    t = np.clip(np.einsum("nd,nd->n", xn, wn_lab), -1.0, 1.0)

    xq_f = xq.astype(np.float32).astype(np.float64)
    wq_lab = wq[labels].astype(np.float32).astype(np.float64)
    t_q = np.einsum("nd,nd->n", xq_f, wq_lab) / (FS * FS)

    m = np.where(labels <= 5, ML, MS)
    num = S * (t - m)
    excl = total - np.exp(S * t_q)
    L = num - np.log(np.exp(num) + excl)
    return np.float32(-L.mean())


# revision 9
# speedup vs baseline: 1.8216x; 1.0462x over previous
"""AdMSoftmaxLoss distributed Trainium2 kernel (v2 — host-prepped operands).

Reference computation (N=8192, D=1024, C=10240, S=30, ml=0.4, ms=0.1):
    wf    = clip(l2norm(x) @ l2norm(weight).T, -1, 1)      # (N, C) cosines
    m     = where(labels <= 5, ml, ms)
    t     = wf[i, labels[i]]
    num   = S * (t - m)
    excl  = sum_j exp(S * wf[i, j]) - exp(S * t)
    L     = num - log(exp(num) + excl)
    loss  = -mean(L)

Sharding: 2 row-groups x 4 class-groups over 8 NeuronCores. Core i gets
rows [ (i//4)*4096, .. ) and classes [ (i%4)*2560, .. ).

Division of labor:
  - HOST (numpy, ~1e7 elem ops, 1e4x less work than the device matmul):
    l2-normalize x and weight, scale by 16, cast to fp8e4m3, and lay the
    operands out d-major (pre-transposed) exactly as the PE wants them.
    Also computes the per-row label term t = cos(x_i, w_label) exactly,
    which replaces both the device-side label gather and the all-reduce.
  - DEVICE: for its (4096 rows x 2560 classes) block, computes
    out[p, m] = sum_c exp(S * cos[row, c]) via fp8 DoubleRow matmuls
    (contraction 256/pass, 4 passes over D=1024) and ScalarE Exp with
    fused row-sum accumulation. That is the only O(N*C) work.
  - HOST finish: total denominator = sum of 4 class-group partials,
    excl = total - exp(S*t_q), L = num - log(exp(num) + excl), mean.

Device pipeline per core: DMA fp8 operands in 7 chunks ordered so the
first matmul can start after ~1.5MB has landed; 640 DoubleRow matmuls
(N=512 each, PSUM f32, 3 rotating 2-bank accumulators); 96 Exp
activations with accum_out; one tensor_reduce; 16KB output DMA.
"""

import math
import os
import numpy as np

P = 128
N_ROWS, D, C = 8192, 1024, 10240
S = 30.0
ML, MS = 0.4, 0.1
NCORES = 8
RG, CG = 2, 4                  # row groups x class groups
R_LOC = N_ROWS // RG           # 4096
C_LOC = C // CG                # 2560
M_TILES = R_LOC // P           # 32
K_TILES = D // P               # 8
KP = K_TILES // 2              # 4 DoubleRow passes (256 contraction each)
XCH = 4                        # x row chunks (1024 rows each)
XW = R_LOC // XCH              # 1024
G_MT = XW // P                 # 8 m-tiles per x chunk
SUPER = [(0, 512), (512, 1024), (1536, 1024)]   # class superchunks
NSC = len(SUPER)
FS = 16.0                      # fp8 pre-scale on both operands
EXPSCALE = S / (FS * FS)       # PSUM holds FS^2 * cos

_CACHE = {}
LAST_RESULTS = None  # BassKernelResults of the most recent run (for test.py)


def _build():
    """Build + compile the SPMD Bass graph once; cache in module global."""
    if "nc" in _CACHE:
        return _CACHE["nc"]

    import concourse.bass as bass
    import concourse.mybir as mybir
    import concourse.tile as tile
    from concourse import bacc

    ts = bass.ts
    dt = mybir.dt
    AF = mybir.ActivationFunctionType
    ALU = mybir.AluOpType

    nc = bacc.Bacc(
        "TRN2", target_bir_lowering=False, debug=False, num_devices=NCORES
    )

    x_ext = nc.dram_tensor(
        "xq", [P, K_TILES, R_LOC], dt.float8e4, kind="ExternalInput"
    ).ap()
    w_exts = [
        nc.dram_tensor(
            f"wq{si}", [P, K_TILES, w], dt.float8e4, kind="ExternalInput"
        ).ap()
        for si, (c0, w) in enumerate(SUPER)
    ]
    out_ext = nc.dram_tensor(
        "out", [XCH, P, G_MT, NSC], dt.float32, kind="ExternalOutput"
    ).ap()

    with tile.TileContext(nc) as tc:
        with (
            tc.tile_pool(name="consts", bufs=1) as consts,
            tc.tile_pool(name="esc", bufs=3) as escp,
            tc.tile_pool(name="psum", bufs=3, space="PSUM") as psum,
            tc.tile_pool(name="psumw", bufs=1, space="PSUM") as psumw,
        ):
            wsb = [
                consts.tile([P, K_TILES, w], dt.float8e4, name=f"w{si}", tag=f"w{si}")
                for si, (c0, w) in enumerate(SUPER)
            ]
            xsb = consts.tile([P, K_TILES, R_LOC], dt.float8e4, name="xsb")
            sums = [
                consts.tile([P, G_MT, NSC], dt.float32, name=f"s{g}", tag=f"s{g}")
                for g in range(XCH)
            ]

            # One HWDGE ring, FIFO: w chunk 0, x rows 0-511 (gates the
            # first matmuls), x rows 512-1023, later w chunks, x rest as
            # a single large transfer (sub-slice deps let row group g wait
            # only on the bytes it reads).
            nc.sync.dma_start(wsb[0][:], w_exts[0])
            nc.sync.dma_start(xsb[:, :, 0:512], x_ext[:, :, 0:512])
            nc.sync.dma_start(xsb[:, :, 512:XW], x_ext[:, :, 512:XW])
            nc.sync.dma_start(wsb[1][:], w_exts[1])
            nc.sync.dma_start(wsb[2][:], w_exts[2])
            nc.sync.dma_start(xsb[:, :, XW:R_LOC], x_ext[:, :, XW:R_LOC])

            # Warm the PE HAM clock gate while the first chunks stream in:
            # a dozen throwaway matmuls on a zeroed tile, so the first
            # real matmuls run at 2.4 GHz instead of 1.2.
            zf = consts.tile([P, 2, 512], dt.float8e4)
            nc.gpsimd.memset(zf[:], 0.0)
            zps = psumw.tile([P, 512], dt.float32)
            for _ in range(12):
                nc.tensor.matmul(
                    zps[:],
                    zf[:, :, 0:P],
                    zf[:],
                    start=True,
                    stop=True,
                    perf_mode=mybir.MatmulPerfMode.DoubleRow,
                )

            def block(g, jj, si):
                """Matmuls + exp row-sum for (m-tile, superchunk)."""
                m = g * G_MT + jj
                _, w = SUPER[si]
                ps = psum.tile([P, 1024], dt.float32, tag="ps")
                for kp in range(KP):
                    for h in range(w // 512):
                        nc.tensor.matmul(
                            ps[:, ts(h, 512)],
                            xsb[:, 2 * kp : 2 * kp + 2, ts(m, P)],
                            wsb[si][:, 2 * kp : 2 * kp + 2, ts(h, 512)],
                            start=(kp == 0),
                            stop=(kp == KP - 1),
                            perf_mode=mybir.MatmulPerfMode.DoubleRow,
                        )
                esc = escp.tile([P, 1024], dt.bfloat16, tag="esc")
                nc.scalar.activation(
                    esc[:, :w],
                    ps[:, :w],
                    AF.Exp,
                    scale=EXPSCALE,
                    accum_out=sums[g][:, jj, si : si + 1],
                )

            for g in range(XCH):
                if g == 0:
                    # si-major startup: row chunk 0 runs all 8 m-tiles of
                    # superchunk 0 first (jj 0-3 before 4-7 so only the
                    # first 512 rows of xq gate the first matmul).
                    for si in range(NSC):
                        for jj in range(G_MT):
                            block(g, jj, si)
                else:
                    for jj in range(G_MT):
                        # narrow superchunk last on the final m-tile so
                        # the very last Exp is the short one
                        order = (1, 2, 0) if (g == XCH - 1 and jj == G_MT - 1) else (0, 1, 2)
                        for si in order:
                            block(g, jj, si)
                # flush this row group's partials; host adds the three
                # superchunk columns
                nc.sync.dma_start(out_ext[g], sums[g][:])

    nc.compile()
    _CACHE["nc"] = nc
    return nc


def _prep_inputs(x, weight):
    """Normalize, scale, fp8-quantize, and transpose operands host-side.

    Returns (in_maps_pieces, xq_f32, wq_f32) where the f32 copies of the
    quantized values are used to reproduce the device's label term.
    """
    import ml_dtypes

    f8 = ml_dtypes.float8_e4m3

    xn = x / np.maximum(np.sqrt((x * x).sum(1, keepdims=True)), 1e-12)
    wn = weight / np.maximum(np.sqrt((weight * weight).sum(1, keepdims=True)), 1e-12)
    xq = (xn * FS).astype(f8)
    wq = (wn * FS).astype(f8)

    x_groups = []
    for gr in range(RG):
        xg = xq[gr * R_LOC : (gr + 1) * R_LOC]          # [4096, 1024]
        # A[p, k, r] = xg.T[k*128+p, r]
        a = np.ascontiguousarray(
            xg.T.reshape(K_TILES, P, R_LOC).transpose(1, 0, 2)
        )
        x_groups.append(a)

    w_chunks = []
    for ci in range(CG):
        wc = wq[ci * C_LOC : (ci + 1) * C_LOC]           # [2560, 1024]
        wt = wc.T.reshape(K_TILES, P, C_LOC).transpose(1, 0, 2)  # [p, k, c]
        w_chunks.append(
            [np.ascontiguousarray(wt[:, :, c0 : c0 + w]) for c0, w in SUPER]
        )

    return x_groups, w_chunks, xq, wq


def kernel(x, labels, weight):
    global LAST_RESULTS
    from concourse.bass_utils import run_bass_kernel_spmd

    x = np.asarray(x, dtype=np.float32)
    weight = np.asarray(weight, dtype=np.float32)
    labels = np.asarray(labels).astype(np.int64)

    nc = _build()
    x_groups, w_chunks, xq, wq = _prep_inputs(x, weight)

    in_maps = []
    for i in range(NCORES):
        gr, ci = divmod(i, CG)
        im = {"xq": x_groups[gr]}
        for si in range(NSC):
            im[f"wq{si}"] = w_chunks[ci][si]
        in_maps.append(im)

    trace = bool(int(os.environ.get("ADMS_TRACE", "0")))
    res = run_bass_kernel_spmd(nc, in_maps, list(range(NCORES)), trace=trace)
    LAST_RESULTS = res

    total = np.zeros(N_ROWS, np.float64)
    for i, r in enumerate(res.results):
        gr = i // CG
        o = np.asarray(r["out"], dtype=np.float64)       # [4, 128, 8, 3]
        # row = g*1024 + jj*128 + p  ->  [g, jj, p] flat; sum superchunks
        part = o.sum(-1).transpose(0, 2, 1).reshape(R_LOC)
        total[gr * R_LOC : (gr + 1) * R_LOC] += part

    # Label term: exact for the numerator; quantized (matching the
    # device's fp8 operands) for the excl subtraction.
    xn = x.astype(np.float64)
    xn /= np.maximum(np.sqrt((xn * xn).sum(1, keepdims=True)), 1e-12)
    wn_lab = weight[labels].astype(np.float64)
    wn_lab /= np.maximum(np.sqrt((wn_lab * wn_lab).sum(1, keepdims=True)), 1e-12)
    t = np.clip(np.einsum("nd,nd->n", xn, wn_lab), -1.0, 1.0)

    xq_f = xq.astype(np.float32).astype(np.float64)
    wq_lab = wq[labels].astype(np.float32).astype(np.float64)
    t_q = np.einsum("nd,nd->n", xq_f, wq_lab) / (FS * FS)

    m = np.where(labels <= 5, ML, MS)
    num = S * (t - m)
    excl = total - np.exp(S * t_q)
    L = num - np.log(np.exp(num) + excl)
    return np.float32(-L.mean())
